# revision 1
# baseline (speedup 1.0000x reference)
"""Trainium2 Bass kernel for nn_Attention_14542759264705.

Dense transformer attention: QKV proj + interleaved RoPE + GQA causal
attention (32 q heads / 8 kv heads, hd=64) + output proj, fp32 in/out.

Sharding: tensor-parallel over kv-head groups across 8 cores. Core c owns
q heads 4c..4c+3 and kv head c; each core computes a partial output and
the host sums the 8 partials.

All matmuls run in float32r (tf32-class, ~2e-4 end-to-end). The PE
instruction stream is kept dense (it executes in order, and back-to-back
matmuls hide their weight loads):
  Phase 1 per 512-token chunk: DMA x, PE-transpose to xT, 3-chunk QKV
  projection accumulating over d; RoPE on DVE in a host-permuted
  channel layout where real/imag parts are partition-aligned; Q written
  to per-head [r;i] layout via ACT partition-shift copies (so scores
  contract K=64 in a single matmul), K duplicated to partition groups
  0/64, V transposed to [token, ch] (+ones column -> PV matmul also
  produces the softmax denominator). V transposes for chunk j-1 are
  interleaved into chunk j so they never block the PE.
  Phase 2 per (batch, 512-query pair), two head-pair passes over the
  causal kt range: one K=64 scoresT matmul per head (row groups 0/64
  concurrent), exp on ACT (scale=1/8 folded; no max subtraction needed,
  |logits| < ~6), diagonal masks on DVE, PV accumulation in PSUM
  [65, 512] lagged one kt behind scores; reciprocal + gpsimd
  partition-broadcast normalization; wo matmuls + output DMA lagged one
  pair behind. Mostly-masked diagonal kt tiles use half-width
  scores/exp/PV.
"""
import numpy as np

B, S, D = 2, 2048, 2048
T = B * S
NH, NKV, HD = 32, 8, 64
NCORES = 8

_cache = {}


def _build(phases=99):
    import concourse.bacc as bacc
    import concourse.mybir as mybir
    import concourse.tile as tile
    from concourse.masks import make_identity

    F32 = mybir.dt.float32
    F32R = mybir.dt.float32r
    AF = mybir.ActivationFunctionType

    nc = bacc.Bacc("TRN2", target_bir_lowering=False, debug=False,
                   num_devices=NCORES)
    x = nc.dram_tensor("x", [T, D], F32, kind="ExternalInput").ap()
    wqkvT = nc.dram_tensor("wqkvT", [D, 384], F32, kind="ExternalInput").ap()
    woT = nc.dram_tensor("woT", [256, D], F32, kind="ExternalInput").ap()
    c4 = nc.dram_tensor("c4", [128, S], F32, kind="ExternalInput").ap()
    s4 = nc.dram_tensor("s4", [128, S], F32, kind="ExternalInput").ap()
    maskP = nc.dram_tensor("maskP", [128, 4 * 512], F32,
                           kind="ExternalInput").ap()
    o = nc.dram_tensor("o", [T, D], F32, kind="ExternalOutput").ap()

    with tile.TileContext(nc) as tc:
        with tc.tile_pool(name="resident", bufs=1) as res:
            ident = res.tile([128, 128], F32)
            make_identity(nc, ident[:])
            c4_sb = res.tile([128, S], F32)
            s4_sb = res.tile([128, S], F32)
            maskP_sb = res.tile([128, 4 * 512], F32)
            nc.sync.dma_start(c4_sb[:], c4[:])
            nc.sync.dma_start(s4_sb[:], s4[:])
            nc.sync.dma_start(maskP_sb[:], maskP[:])

            QRI_A = res.tile([128, T], F32R)   # [h0r h0i h1r h1i] x tokens
            QRI_B = res.tile([128, T], F32R)   # [h2r h2i h3r h3i]
            KRI2 = res.tile([128, T], F32R)    # [Kr Ki Kr Ki]
            Vt_sb = res.tile([128, 32 * 65], F32R)   # kt-tile k at cols k*65
            Vt3 = Vt_sb.rearrange("p (k c) -> p k c", c=65)
            woT_r = res.tile([128, 2 * D], F32R)
            ones32 = res.tile([128, 32], F32)
            nc.gpsimd.memset(ones32[:], 1.0)
            nc.vector.tensor_copy(Vt3[:, :, 64], ones32[:])

            # ---------------- phase 1: weights, xT, proj, rope --------------
            with tc.tile_pool(name="wqp", bufs=1) as wqp, \
                 tc.tile_pool(name="xstage", bufs=1) as xst, \
                 tc.tile_pool(name="xTp", bufs=5) as xTp, \
                 tc.tile_pool(name="ropet", bufs=2) as rp, \
                 tc.tile_pool(name="trps", bufs=2, space="PSUM") as trp_pool, \
                 tc.tile_pool(name="vtps", bufs=1, space="PSUM") as vtp_pool, \
                 tc.tile_pool(name="projps", bufs=1, space="PSUM") as projp:

                wqkv_r = wqp.tile([128, 16 * 384], F32R)
                for d in range(16):
                    wst = xst.tile([128, 384], F32, name=f"xh{d % 4}")
                    nc.sync.dma_start(wst[:], wqkvT[d * 128:(d + 1) * 128, :])
                    nc.vector.tensor_copy(
                        wqkv_r[:, d * 384:(d + 1) * 384], wst[:])
                for t in range(2):
                    for h2 in range(2):
                        ws = xst.tile([128, 1024], F32,
                                      name=f"xh{4 + 2 * t + h2}")
                        nc.sync.dma_start(
                            ws[:], woT[t * 128:(t + 1) * 128,
                                       h2 * 1024:(h2 + 1) * 1024])
                        nc.vector.tensor_copy(
                            woT_r[:, t * D + h2 * 1024:
                                  t * D + (h2 + 1) * 1024], ws[:])

                vsb_prev = None

                def emit_vt(jj, vsb):
                    for i in range(4):
                        vtp = vtp_pool.tile([128, 64], F32, name="vtp")
                        nc.tensor.transpose(
                            vtp[:], vsb[:, i * 128:(i + 1) * 128],
                            ident[0:64, 0:64])
                        nc.scalar.copy(Vt3[:, jj * 4 + i, 0:64], vtp[:])

                for j in range(8):           # 512-token chunks
                    xh = {}
                    for h in range(2):
                        for i in range(4):
                            xt = xst.tile(
                                [128, 1024], F32,
                                name=f"xh{((j * 2 + h) % 3) * 4 + i}")
                            nc.sync.dma_start(
                                xt[:],
                                x[j * 512 + i * 128:j * 512 + (i + 1) * 128,
                                  h * 1024:(h + 1) * 1024])
                            xh[(h, i)] = xt
                    QRp = projp.tile([128, 512], F32, name="QRp", bufs=2)
                    QIp = projp.tile([128, 512], F32, name="QIp", bufs=2)
                    KVp = projp.tile([128, 512], F32, name="KVp", bufs=1)
                    for d in range(16):
                        hf, dl = d // 8, d % 8
                        xTd = xTp.tile([128, 512], F32R, name="xTd")
                        trp = trp_pool.tile([128, 512], F32, name="trp")
                        for i in range(4):
                            nc.tensor.transpose(
                                trp[:, i * 128:(i + 1) * 128],
                                xh[(hf, i)][:, dl * 128:(dl + 1) * 128],
                                ident[:])
                        nc.scalar.copy(xTd[:], trp[:])
                        for ch, ps in enumerate((QRp, QIp, KVp)):
                            nc.tensor.matmul(
                                ps[:],
                                wqkv_r[:, d * 384 + ch * 128:
                                       d * 384 + (ch + 1) * 128],
                                xTd[:], start=(d == 0), stop=(d == 15))
                        if d == 8 and vsb_prev is not None:
                            emit_vt(j - 1, vsb_prev)
                    # rope (emission order frees proj psum bufs asap:
                    # vsb + K-rope release KVp, t1/t3 release QRp)
                    tb = j * 512
                    bc = (j % 4) * 512
                    cs = c4_sb[:, bc:bc + 512]
                    sn = s4_sb[:, bc:bc + 512]
                    cs32 = c4_sb[0:32, bc:bc + 512]
                    sn32 = s4_sb[0:32, bc:bc + 512]
                    vsb = rp.tile([64, 512], F32, name="vsb")
                    nc.scalar.copy(vsb[:], KVp[64:128, :])
                    u1 = rp.tile([32, 512], F32, name="u1", bufs=1)
                    u2 = rp.tile([32, 512], F32, name="u2", bufs=1)
                    u3 = rp.tile([32, 512], F32, name="u3", bufs=1)
                    u4 = rp.tile([32, 512], F32, name="u4", bufs=1)
                    nc.vector.tensor_mul(u1[:], KVp[0:32, :], cs32)
                    nc.vector.tensor_mul(u2[:], KVp[32:64, :], sn32)
                    nc.vector.tensor_mul(u3[:], KVp[0:32, :], sn32)
                    nc.vector.tensor_mul(u4[:], KVp[32:64, :], cs32)
                    t1 = rp.tile([128, 512], F32, name="t1", bufs=1)
                    t2 = rp.tile([128, 512], F32, name="t2", bufs=1)
                    t3 = rp.tile([128, 512], F32, name="t3", bufs=1)
                    t4 = rp.tile([128, 512], F32, name="t4", bufs=1)
                    qtr = rp.tile([128, 512], F32, name="qtr")
                    qti = rp.tile([128, 512], F32, name="qti")
                    nc.vector.tensor_mul(t1[:], QRp[:], cs)
                    nc.vector.tensor_mul(t3[:], QRp[:], sn)
                    nc.vector.tensor_mul(t2[:], QIp[:], sn)
                    nc.vector.tensor_mul(t4[:], QIp[:], cs)
                    for g in (0, 64):
                        nc.vector.tensor_sub(
                            KRI2[g:g + 32, tb:tb + 512], u1[:], u2[:])
                    for g in (32, 96):
                        nc.vector.tensor_add(
                            KRI2[g:g + 32, tb:tb + 512], u3[:], u4[:])
                    nc.vector.tensor_sub(qtr[:], t1[:], t2[:])
                    nc.vector.tensor_add(qti[:], t3[:], t4[:])
                    for hh in range(4):
                        dst = QRI_A if hh < 2 else QRI_B
                        base = (hh % 2) * 64
                        nc.scalar.copy(
                            dst[base:base + 32, tb:tb + 512],
                            qtr[32 * hh:32 * hh + 32, :])
                        nc.scalar.copy(
                            dst[base + 32:base + 64, tb:tb + 512],
                            qti[32 * hh:32 * hh + 32, :])
                    vsb_prev = vsb
                emit_vt(7, vsb_prev)

            if phases < 2:
                nc.sync.dma_start(
                    o[0:128, :], QRI_A[:, 0:2048].bitcast(F32))
            # -------------- phase 2: attention + wo, per qt-512 pair --------
            else:
                with tc.tile_pool(name="probs", bufs=3) as probsp, \
                     tc.tile_pool(name="attnp", bufs=2) as attnp, \
                     tc.tile_pool(name="normp", bufs=2) as normp, \
                     tc.tile_pool(name="outp", bufs=2) as outp, \
                     tc.tile_pool(name="sps", bufs=2, space="PSUM") as sps, \
                     tc.tile_pool(name="pvps", bufs=1, space="PSUM") as pvps, \
                     tc.tile_pool(name="ops", bufs=2, space="PSUM") as opsp:

                    def emit_wo(attn01, attn23, qb):
                        for qs in range(4):
                            qq = qb + qs * 128
                            osb = outp.tile([128, D], F32, name="osb")
                            for do in range(4):
                                Ops = opsp.tile([128, 512], F32, name="Ops")
                                nc.tensor.matmul(
                                    Ops[:],
                                    attn01[:, qs * 128:(qs + 1) * 128],
                                    woT_r[:, do * 512:(do + 1) * 512],
                                    start=True, stop=False)
                                nc.tensor.matmul(
                                    Ops[:],
                                    attn23[:, qs * 128:(qs + 1) * 128],
                                    woT_r[:, D + do * 512:
                                          D + (do + 1) * 512],
                                    start=False, stop=True)
                                nc.vector.tensor_copy(
                                    osb[:, do * 512:(do + 1) * 512], Ops[:])
                            nc.sync.dma_start(o[qq:qq + 128, :], osb[:])

                    wo_prev = None
                    for b in range(2):
                        for jp in range(4):          # qt-512 pairs
                            qb = b * S + jp * 512
                            nkt = 4 * jp + 4
                            attn01 = attnp.tile([128, 512], F32R, name="at01")
                            attn23 = attnp.tile([128, 512], F32R, name="at23")
                            for pi, (QRI, attn) in enumerate(
                                    ((QRI_A, attn01), (QRI_B, attn23))):
                                PVs = [pvps.tile([65, 512], F32,
                                                 name=f"PV{hh}")
                                       for hh in range(2)]
                                pg_prev = None
                                for kt in range(nkt):
                                    kc = b * S + kt * 128
                                    r = kt - (nkt - 4)
                                    half = r >= 2   # only right half live
                                    csl = slice(256, 512) if half \
                                        else slice(0, 512)
                                    Sg = [sps.tile([128, 512], F32,
                                                   name=f"S{hh}")
                                          for hh in range(2)]
                                    pg = [probsp.tile([128, 512], F32R,
                                                      name=f"p{hh}")
                                          for hh in range(2)]
                                    for hh in range(2):
                                        nc.tensor.matmul(
                                            Sg[hh][:, csl],
                                            KRI2[64 * hh:64 * hh + 64,
                                                 kc:kc + 128],
                                            QRI[64 * hh:64 * hh + 64,
                                                qb + csl.start:
                                                qb + csl.stop],
                                            start=True, stop=True,
                                            tile_position=(64 * hh, 0))
                                    for hh in range(2):
                                        nc.scalar.activation(
                                            pg[hh][:, csl], Sg[hh][:, csl],
                                            AF.Exp, scale=0.125)
                                    if r >= 0:
                                        msl = slice(512 * r + 256,
                                                    512 * r + 512) if half \
                                            else slice(512 * r, 512 * r + 256)
                                        psl = slice(256, 512) if half \
                                            else slice(0, 256)
                                        for hh in range(2):
                                            nc.vector.tensor_mul(
                                                pg[hh][:, psl],
                                                pg[hh][:, psl],
                                                maskP_sb[:, msl])
                                    if pg_prev is not None:
                                        pkt, ppg, pcsl = pg_prev
                                        vt = Vt3[:, b * 16 + pkt, :]
                                        st = (pkt == 0)
                                        for hh in range(2):
                                            nc.tensor.matmul(
                                                PVs[hh][:, pcsl], vt,
                                                ppg[hh][:, pcsl],
                                                start=st, stop=False)
                                    pg_prev = (kt, pg, csl)
                                pkt, ppg, pcsl = pg_prev
                                vt = Vt3[:, b * 16 + pkt, :]
                                for hh in range(2):
                                    nc.tensor.matmul(
                                        PVs[hh][:, pcsl], vt,
                                        ppg[hh][:, pcsl],
                                        start=(pkt == 0), stop=True)
                                pvc = [normp.tile([65, 512], F32,
                                                  name=f"pvc{hh}")
                                       for hh in range(2)]
                                for hh in range(2):
                                    nc.vector.tensor_copy(
                                        pvc[hh][:], PVs[hh][:])
                                for hh in range(2):
                                    rec = normp.tile([1, 512], F32,
                                                     name=f"rec{hh}")
                                    nc.vector.reciprocal(
                                        rec[:], pvc[hh][64:65, :])
                                    bcst = normp.tile([64, 512], F32,
                                                      name=f"bc{hh}")
                                    nc.gpsimd.partition_broadcast(
                                        bcst[:], rec[:])
                                    nc.vector.tensor_mul(
                                        attn[64 * hh:64 * hh + 64, :],
                                        pvc[hh][0:64, :], bcst[:])
                                if pi == 0 and wo_prev is not None:
                                    emit_wo(*wo_prev)
                                    wo_prev = None
                            wo_prev = (attn01, attn23, qb)
                    emit_wo(*wo_prev)

    nc.compile()
    return nc


def _prep_inputs(x, freqs_cos, freqs_sin, wq, wk, wv, wo):
    xf = np.ascontiguousarray(np.asarray(x, np.float32).reshape(T, D))
    wq = np.asarray(wq, np.float32)
    wk = np.asarray(wk, np.float32)
    wv = np.asarray(wv, np.float32)
    wo = np.asarray(wo, np.float32)
    fc = np.asarray(freqs_cos, np.float32)
    fs = np.asarray(freqs_sin, np.float32)

    c4 = np.ascontiguousarray(np.tile(fc.T, (4, 1)))       # [128, S]
    s4 = np.ascontiguousarray(np.tile(fs.T, (4, 1)))
    kt = np.arange(128)[:, None]
    qt = np.arange(256)[None, :]
    mA = (kt <= qt).astype(np.float32)
    mB = (kt + 128 <= qt).astype(np.float32)
    one = np.ones((128, 256), np.float32)
    zero = np.zeros((128, 256), np.float32)
    maskP = np.concatenate([
        np.concatenate([mA, one], axis=1),
        np.concatenate([mB, one], axis=1),
        np.concatenate([zero, mA], axis=1),
        np.concatenate([zero, mB], axis=1)], axis=1)       # [128, 2048]
    ev = np.arange(0, 64, 2)
    od = np.arange(1, 64, 2)

    in_maps = []
    for c in range(NCORES):
        qreal = np.concatenate([(4 * c + h) * 64 + ev for h in range(4)])
        qimag = np.concatenate([(4 * c + h) * 64 + od for h in range(4)])
        Wc = np.concatenate([wq[qreal], wq[qimag], wk[c * 64 + ev],
                             wk[c * 64 + od], wv[c * 64:(c + 1) * 64]], axis=0)
        in_maps.append({
            "x": xf,
            "wqkvT": np.ascontiguousarray(Wc.T),
            "woT": np.ascontiguousarray(wo[:, c * 256:(c + 1) * 256].T),
            "c4": c4, "s4": s4, "maskP": maskP,
        })
    return in_maps


def _run(in_maps, trace=False, **kw):
    from concourse import bass_utils
    if "nc" not in _cache:
        _cache["nc"] = _build()
    return bass_utils.run_bass_kernel_spmd(
        _cache["nc"], in_maps, core_ids=list(range(NCORES)), trace=trace, **kw)


def kernel(x, freqs_cos, freqs_sin, wq, wk, wv, wo):
    in_maps = _prep_inputs(x, freqs_cos, freqs_sin, wq, wk, wv, wo)
    res = _run(in_maps)
    out = res.results[0]["o"].astype(np.float64)
    for c in range(1, NCORES):
        out += res.results[c]["o"]
    return out.astype(np.float32).reshape(B, S, D)



# revision 26
# speedup vs baseline: 1.6566x; 1.6566x over previous
"""Trainium2 Bass kernel for nn_Attention_14542759264705.

Dense transformer attention: QKV proj + interleaved RoPE + GQA causal
attention (32 q heads / 8 kv heads, hd=64) + output proj, fp32 in/out.

Sharding: tensor-parallel over kv-head groups across 8 cores. Core c owns
q heads 4c..4c+3 and kv head c; each core computes a partial output and
the host sums the 8 partials.

v2 (vs the fp32r baseline):
  - x is transposed on the HOST (xT input) -> no PE transposes / ACT
    copies for the projection's moving operand.
  - All matmul operands are bf16 (PSUM accumulation stays f32): halves
    DMA volume, enables fast weight load, 2x DVE on 16-bit tiles. fp32r
    at N>=256 is already 1 cyc/row, so MM time is unchanged; the wins
    are bandwidth + LDWEIGHTS + elementwise.
  - The two per-head-pair exps are merged into one [128,1024] ACT call
    (amortizes the ~352-cycle ACT fixed cost; ACT exp is the phase-2
    pace-setter).
  - Softmax normalization: DVE reciprocal_approx_fast on the [1,1024]
    denominator row (the old nc.vector.reciprocal was 8 cyc/elem),
    gpsimd partition-broadcast, DVE muls. All off the PE critical path.
  - wo matmuls are drip-fed one (qs,do) step per kt-iteration into the
    NEXT pair's attention loop so the PE never parks while ACT works,
    instead of a 7us wo burst that starves ACT.
"""
import numpy as np

B, S, D = 2, 2048, 2048
T = B * S
NH, NKV, HD = 32, 8, 64
NCORES = 8

_cache = {}


def _build(phases=99):
    from collections import deque

    import concourse.bacc as bacc
    import concourse.mybir as mybir
    import concourse.tile as tile
    from concourse.masks import make_identity

    F32 = mybir.dt.float32
    BF16 = mybir.dt.bfloat16
    AF = mybir.ActivationFunctionType

    nc = bacc.Bacc("TRN2", target_bir_lowering=False, debug=False,
                   num_devices=NCORES)
    xT = nc.dram_tensor("xT", [D, T], BF16, kind="ExternalInput").ap()
    wqkvT = nc.dram_tensor("wqkvT", [D, 384], BF16, kind="ExternalInput").ap()
    woT = nc.dram_tensor("woT", [256, D], BF16, kind="ExternalInput").ap()
    c4 = nc.dram_tensor("c4", [128, S], F32, kind="ExternalInput").ap()
    s4 = nc.dram_tensor("s4", [128, S], F32, kind="ExternalInput").ap()
    maskP = nc.dram_tensor("maskP", [128, 2 * 4 * 512], BF16,
                           kind="ExternalInput").ap()
    o = nc.dram_tensor("o", [T, D], BF16, kind="ExternalOutput").ap()

    with tile.TileContext(nc) as tc:
        with tc.tile_pool(name="resident", bufs=1) as res:
            ident64 = res.tile([64, 64], BF16)
            make_identity(nc, ident64[:])
            c4_sb = res.tile([128, S], F32)
            s4_sb = res.tile([128, S], F32)
            maskP_sb = res.tile([128, 2 * 4 * 512], BF16)
            nc.sync.dma_start(c4_sb[:], c4[:])
            nc.sync.dma_start(s4_sb[:], s4[:])
            nc.sync.dma_start(maskP_sb[:], maskP[:])

            QRI_A = res.tile([128, T], BF16)   # [h0r h0i h1r h1i] x tokens
            QRI_B = res.tile([128, T], BF16)   # [h2r h2i h3r h3i]
            KRI2 = res.tile([128, T], BF16)    # [Kr Ki Kr Ki]
            Vt_sb = res.tile([128, 32 * 65], BF16)  # kt-tile k at cols k*65
            Vt3 = Vt_sb.rearrange("p (k c) -> p k c", c=65)
            wqkv_r = res.tile([128, 16 * 384], BF16)
            woT_r = res.tile([128, 2 * D], BF16)
            ones32 = res.tile([128, 32], BF16)
            nc.gpsimd.memset(ones32[:], 1.0)
            nc.vector.tensor_copy(Vt3[:, :, 64], ones32[:])
            dbg_pg = (res.tile([128, 1024], BF16, name="dbg_pg")
                      if phases == 3 else None)

            for d in range(16):
                nc.sync.dma_start(wqkv_r[:, d * 384:(d + 1) * 384],
                                  wqkvT[d * 128:(d + 1) * 128, :])
            for t in range(2):
                nc.sync.dma_start(woT_r[:, t * D:(t + 1) * D],
                                  woT[t * 128:(t + 1) * 128, :])

            # ---------------- phase 1: xT DMA, proj, rope -------------------
            with tc.tile_pool(name="xtp", bufs=3) as xtp, \
                 tc.tile_pool(name="ropet", bufs=2) as rp, \
                 tc.tile_pool(name="vtps", bufs=1, space="PSUM") as vtp_pool, \
                 tc.tile_pool(name="projps", bufs=1, space="PSUM") as projp:

                vsb_prev = None

                def emit_vt(jj, vsb):
                    vtp = vtp_pool.tile([128, 256], BF16, name="vtp")
                    for i in range(4):
                        nc.tensor.transpose(
                            vtp[:, i * 64:(i + 1) * 64],
                            vsb[:, i * 128:(i + 1) * 128], ident64[:])
                    vtp3 = vtp.rearrange("p (k c) -> p k c", c=64)
                    nc.vector.tensor_copy(
                        Vt3[:, jj * 4:jj * 4 + 4, 0:64], vtp3[:])

                for j in range(8):           # 512-token chunks
                    xts = []
                    for d in range(16):
                        xt = xtp.tile([128, 512], BF16, name=f"xt{d}")
                        nc.sync.dma_start(
                            xt[:], xT[d * 128:(d + 1) * 128,
                                      j * 512:(j + 1) * 512])
                        xts.append(xt)
                    QRp = projp.tile([128, 512], F32, name="QRp", bufs=2)
                    QIp = projp.tile([128, 512], F32, name="QIp", bufs=2)
                    KVp = projp.tile([128, 512], F32, name="KVp", bufs=2)
                    for d in range(16):
                        for ch, ps in enumerate((QRp, QIp, KVp)):
                            nc.tensor.matmul(
                                ps[:],
                                wqkv_r[:, d * 384 + ch * 128:
                                       d * 384 + (ch + 1) * 128],
                                xts[d][:], start=(d == 0), stop=(d == 15))
                        if d == 8 and vsb_prev is not None:
                            emit_vt(j - 1, vsb_prev)
                    # rope (vsb + K-rope first to release KVp asap)
                    tb = j * 512
                    bc = (j % 4) * 512
                    cs = c4_sb[:, bc:bc + 512]
                    sn = s4_sb[:, bc:bc + 512]
                    cs32 = c4_sb[0:32, bc:bc + 512]
                    sn32 = s4_sb[0:32, bc:bc + 512]
                    vsb = rp.tile([64, 512], BF16, name="vsb")
                    nc.scalar.copy(vsb[:], KVp[64:128, :])
                    u1 = rp.tile([32, 512], F32, name="u1", bufs=1)
                    u2 = rp.tile([32, 512], F32, name="u2", bufs=1)
                    u3 = rp.tile([32, 512], F32, name="u3", bufs=1)
                    u4 = rp.tile([32, 512], F32, name="u4", bufs=1)
                    nc.vector.tensor_mul(u1[:], KVp[0:32, :], cs32)
                    nc.vector.tensor_mul(u2[:], KVp[32:64, :], sn32)
                    nc.vector.tensor_mul(u3[:], KVp[0:32, :], sn32)
                    nc.vector.tensor_mul(u4[:], KVp[32:64, :], cs32)
                    for g in (0, 64):
                        nc.gpsimd.tensor_sub(
                            KRI2[g:g + 32, tb:tb + 512], u1[:], u2[:])
                    for g in (32, 96):
                        nc.gpsimd.tensor_add(
                            KRI2[g:g + 32, tb:tb + 512], u3[:], u4[:])
                    t1 = rp.tile([128, 512], F32, name="t1", bufs=1)
                    t2 = rp.tile([128, 512], F32, name="t2", bufs=1)
                    t3 = rp.tile([128, 512], F32, name="t3", bufs=1)
                    t4 = rp.tile([128, 512], F32, name="t4", bufs=1)
                    nc.vector.tensor_mul(t1[:], QRp[:], cs)
                    nc.vector.tensor_mul(t3[:], QRp[:], sn)
                    nc.vector.tensor_mul(t2[:], QIp[:], sn)
                    nc.vector.tensor_mul(t4[:], QIp[:], cs)
                    for hh in range(4):
                        dst = QRI_A if hh < 2 else QRI_B
                        base = (hh % 2) * 64
                        nc.gpsimd.tensor_sub(
                            dst[base:base + 32, tb:tb + 512],
                            t1[32 * hh:32 * hh + 32, :],
                            t2[32 * hh:32 * hh + 32, :])
                        nc.gpsimd.tensor_add(
                            dst[base + 32:base + 64, tb:tb + 512],
                            t3[32 * hh:32 * hh + 32, :],
                            t4[32 * hh:32 * hh + 32, :])
                    vsb_prev = vsb
                emit_vt(7, vsb_prev)

            dbg = {}
            if phases < 2:
                nc.sync.dma_start(o[0:128, :], QRI_A[:, 0:2048])
            # -------------- phase 2: attention + wo, per qt-512 pair --------
            else:
                with tc.tile_pool(name="probs", bufs=3) as probsp, \
                     tc.tile_pool(name="attnp", bufs=2) as attnp, \
                     tc.tile_pool(name="normp", bufs=2) as normp, \
                     tc.tile_pool(name="outp", bufs=2) as outp, \
                     tc.tile_pool(name="sps", bufs=2, space="PSUM") as sps, \
                     tc.tile_pool(name="pvps", bufs=1, space="PSUM") as pvps, \
                     tc.tile_pool(name="ops", bufs=2, space="PSUM") as opsp:

                    pending = deque()

                    def enqueue_wo(attn01, attn23, qb):
                        state = {}
                        for qs in range(4):
                            for do in range(4):
                                pending.append(
                                    (attn01, attn23, qb, qs, do, state))

                    def drain_wo(n):
                        for _ in range(n):
                            if not pending:
                                return
                            attn01, attn23, qb, qs, do, state = \
                                pending.popleft()
                            if do == 0:
                                state[qs] = outp.tile([128, D], BF16,
                                                      name="osb")
                            osb = state[qs]
                            Ops = opsp.tile([128, 512], F32, name="Ops")
                            nc.tensor.matmul(
                                Ops[:], attn01[:, qs * 128:(qs + 1) * 128],
                                woT_r[:, do * 512:(do + 1) * 512],
                                start=True, stop=False)
                            nc.tensor.matmul(
                                Ops[:], attn23[:, qs * 128:(qs + 1) * 128],
                                woT_r[:, D + do * 512:D + (do + 1) * 512],
                                start=False, stop=True)
                            nc.vector.tensor_copy(
                                osb[:, do * 512:(do + 1) * 512], Ops[:])
                            if do == 3 and phases != 3:
                                qq = qb + qs * 128
                                nc.sync.dma_start(o[qq:qq + 128, :], osb[:])

                    def emit_pv(PVs, b, pkt, ppg, pcsl, stop):
                        vt = Vt3[:, b * 16 + pkt, :]
                        st = (pkt == 0)
                        for hh in range(2):
                            hs = hh * 512
                            nc.tensor.matmul(
                                PVs[:, hs + pcsl.start:hs + pcsl.stop], vt,
                                ppg[:, hs + pcsl.start:hs + pcsl.stop],
                                start=st, stop=stop)

                    for b in range(2):
                        for jp in range(4):          # qt-512 pairs
                            qb = b * S + jp * 512
                            nkt = 4 * jp + 4
                            attn01 = attnp.tile([128, 512], BF16, name="at01")
                            attn23 = attnp.tile([128, 512], BF16, name="at23")
                            for pi, (QRI, attn) in enumerate(
                                    ((QRI_A, attn01), (QRI_B, attn23))):
                                PVs = pvps.tile([65, 1024], F32, name="PV")
                                pg_prev = None
                                for kt in range(nkt):
                                    kc = b * S + kt * 128
                                    r = kt - (nkt - 4)
                                    half = r >= 2   # only right half live
                                    csl = slice(256, 512) if half \
                                        else slice(0, 512)
                                    Sg = sps.tile([128, 1024], F32, name="Sg")
                                    pg = probsp.tile([128, 1024], BF16,
                                                     name="pg")
                                    for hh in range(2):
                                        hs = hh * 512
                                        nc.tensor.matmul(
                                            Sg[:, hs + csl.start:
                                               hs + csl.stop],
                                            KRI2[64 * hh:64 * hh + 64,
                                                 kc:kc + 128],
                                            QRI[64 * hh:64 * hh + 64,
                                                qb + csl.start:
                                                qb + csl.stop],
                                            start=True, stop=True,
                                            tile_position=(64 * hh, 0))
                                    if half:
                                        sgv = Sg.rearrange(
                                            "p (h c) -> p h c",
                                            h=2)[:, :, 256:512]
                                        pgv = pg.rearrange(
                                            "p (h c) -> p h c",
                                            h=2)[:, :, 256:512]
                                        nc.scalar.activation(
                                            pgv, sgv, AF.Exp, scale=0.125)
                                    else:
                                        nc.scalar.activation(
                                            pg[:], Sg[:], AF.Exp, scale=0.125)
                                    if r >= 0:
                                        if half:
                                            msl = slice(512 * r + 256,
                                                        512 * r + 512)
                                            psl = slice(256, 512)
                                        else:
                                            msl = slice(512 * r,
                                                        512 * r + 256)
                                            psl = slice(0, 256)
                                        pgv = pg.rearrange(
                                            "p (h c) -> p h c", h=2)[:, :, psl]
                                        mkv = maskP_sb.rearrange(
                                            "p (h c) -> p h c", h=2)[:, :, msl]
                                        nc.vector.tensor_mul(pgv, pgv, mkv)
                                    if pg_prev is not None:
                                        emit_pv(PVs, b, *pg_prev, stop=False)
                                    pg_prev = (kt, pg, csl)
                                    if (phases == 3 and b == 1 and jp == 3
                                            and pi == 1 and kt == nkt - 4):
                                        nc.vector.tensor_copy(dbg_pg[:],
                                                              pg[:])
                                    drain_wo(1)
                                emit_pv(PVs, b, *pg_prev, stop=True)
                                # normalization (off PE critical path)
                                pvc = normp.tile([65, 1024], F32, name="pvc")
                                nc.vector.tensor_copy(pvc[:], PVs[:])
                                lnd = normp.tile([1, 1024], F32, name="lnd")
                                nc.scalar.activation(
                                    lnd[:], pvc[64:65, :], AF.Ln)
                                rec = normp.tile([1, 1024], F32, name="rec")
                                nc.scalar.activation(
                                    rec[:], lnd[:], AF.Exp, scale=-1.0)
                                bcst = normp.tile([64, 1024], F32, name="bc")
                                nc.gpsimd.partition_broadcast(bcst[:], rec[:])
                                for hh in range(2):
                                    hs = hh * 512
                                    nc.vector.tensor_mul(
                                        attn[64 * hh:64 * hh + 64, :],
                                        pvc[0:64, hs:hs + 512],
                                        bcst[:, hs:hs + 512])
                                if b == 1 and jp == 3 and pi == 1:
                                    dbg.update(pvc=pvc, rec=rec, bcst=bcst,
                                               pg=pg_prev[1], sg=Sg)
                            enqueue_wo(attn01, attn23, qb)
                            if b == 1 and jp == 3:
                                dbg.update(at01=attn01, at23=attn23)
                    drain_wo(1 << 30)
                    if phases == 3:
                        # debug dumps into sacrificial o rows (bf16)
                        scr = normp.tile([128, 2048], BF16, name="dscr")
                        nc.gpsimd.memset(scr[:], 0.0)
                        nc.sync.dma_start(o[0:128, :], QRI_A[:, 0:2048])
                        nc.sync.dma_start(o[128:256, :], KRI2[:, 0:2048])
                        nc.sync.dma_start(o[256:384, :], Vt_sb[:, 0:2048])
                        nc.vector.tensor_copy(scr[0:65, 0:1024],
                                              dbg["pvc"][:])
                        nc.vector.tensor_copy(scr[0:64, 1024:2048],
                                              dbg["bcst"][:])
                        nc.sync.dma_start(o[384:512, :], scr[:])
                        nc.sync.dma_start(o[512:640, 0:1024], dbg_pg[:])
                        scr3 = normp.tile([128, 2048], BF16, name="dscr3")
                        nc.vector.tensor_copy(scr3[:, 0:512], dbg["at01"][:])
                        nc.vector.tensor_copy(scr3[:, 512:1024],
                                              dbg["at23"][:])
                        nc.sync.dma_start(o[640:768, 0:1024],
                                          scr3[:, 0:1024])

    nc.compile()
    return nc


def _prep_inputs(x, freqs_cos, freqs_sin, wq, wk, wv, wo):
    from ml_dtypes import bfloat16
    xf = np.asarray(x, np.float32).reshape(T, D)
    xTf = np.ascontiguousarray(xf.T).astype(bfloat16)      # [D, T]
    wq = np.asarray(wq, np.float32)
    wk = np.asarray(wk, np.float32)
    wv = np.asarray(wv, np.float32)
    wo = np.asarray(wo, np.float32)
    fc = np.asarray(freqs_cos, np.float32)
    fs = np.asarray(freqs_sin, np.float32)

    c4 = np.ascontiguousarray(np.tile(fc.T, (4, 1)))       # [128, S]
    s4 = np.ascontiguousarray(np.tile(fs.T, (4, 1)))
    kt = np.arange(128)[:, None]
    qt = np.arange(256)[None, :]
    mA = (kt <= qt).astype(np.float32)
    mB = (kt + 128 <= qt).astype(np.float32)
    one = np.ones((128, 256), np.float32)
    zero = np.zeros((128, 256), np.float32)
    maskP1 = np.concatenate([
        np.concatenate([mA, one], axis=1),
        np.concatenate([mB, one], axis=1),
        np.concatenate([zero, mA], axis=1),
        np.concatenate([zero, mB], axis=1)], axis=1)
    maskP = np.ascontiguousarray(
        np.tile(maskP1, (1, 2))).astype(bfloat16)      # [128, 4096]
    ev = np.arange(0, 64, 2)
    od = np.arange(1, 64, 2)

    in_maps = []
    for c in range(NCORES):
        qreal = np.concatenate([(4 * c + h) * 64 + ev for h in range(4)])
        qimag = np.concatenate([(4 * c + h) * 64 + od for h in range(4)])
        Wc = np.concatenate([wq[qreal], wq[qimag], wk[c * 64 + ev],
                             wk[c * 64 + od], wv[c * 64:(c + 1) * 64]], axis=0)
        in_maps.append({
            "xT": xTf,
            "wqkvT": np.ascontiguousarray(Wc.T).astype(bfloat16),
            "woT": np.ascontiguousarray(
                wo[:, c * 256:(c + 1) * 256].T).astype(bfloat16),
            "c4": c4, "s4": s4, "maskP": maskP,
        })
    return in_maps


def _run(in_maps, trace=False, **kw):
    from concourse import bass_utils
    if "nc" not in _cache:
        _cache["nc"] = _build()
    return bass_utils.run_bass_kernel_spmd(
        _cache["nc"], in_maps, core_ids=list(range(NCORES)), trace=trace, **kw)


def kernel(x, freqs_cos, freqs_sin, wq, wk, wv, wo):
    in_maps = _prep_inputs(x, freqs_cos, freqs_sin, wq, wk, wv, wo)
    res = _run(in_maps)
    out = np.zeros((T, D), np.float64)
    for c in range(NCORES):
        out += np.asarray(res.results[c]["o"], np.float32)
    return out.astype(np.float32).reshape(B, S, D)


# revision 40
# speedup vs baseline: 1.9223x; 1.1604x over previous
"""Trainium2 Bass kernel for nn_Attention_14542759264705.

Dense transformer attention: QKV proj + interleaved RoPE + GQA causal
attention (32 q heads / 8 kv heads, hd=64) + output proj, fp32 in/out.

Sharding: tensor-parallel over kv-head groups across 8 cores. Core c owns
q heads 4c..4c+3 and kv head c; each core computes a partial output and
the host sums the 8 partials.

v2 (vs the fp32r baseline):
  - x is transposed on the HOST (xT input) -> no PE transposes / ACT
    copies for the projection's moving operand.
  - All matmul operands are bf16 (PSUM accumulation stays f32): halves
    DMA volume, enables fast weight load, 2x DVE on 16-bit tiles. fp32r
    at N>=256 is already 1 cyc/row, so MM time is unchanged; the wins
    are bandwidth + LDWEIGHTS + elementwise.
  - The two per-head-pair exps are merged into one [128,1024] ACT call
    (amortizes the ~352-cycle ACT fixed cost; ACT exp is the phase-2
    pace-setter).
  - Softmax normalization: DVE reciprocal_approx_fast on the [1,1024]
    denominator row (the old nc.vector.reciprocal was 8 cyc/elem),
    gpsimd partition-broadcast, DVE muls. All off the PE critical path.
  - wo matmuls are drip-fed one (qs,do) step per kt-iteration into the
    NEXT pair's attention loop so the PE never parks while ACT works,
    instead of a 7us wo burst that starves ACT.
"""
import numpy as np

B, S, D = 2, 2048, 2048
T = B * S
NH, NKV, HD = 32, 8, 64
NCORES = 8

_cache = {}


def _build(phases=99):
    from collections import deque

    import concourse.bacc as bacc
    import concourse.mybir as mybir
    import concourse.tile as tile
    from concourse.masks import make_identity

    F32 = mybir.dt.float32
    BF16 = mybir.dt.bfloat16
    AF = mybir.ActivationFunctionType

    # Force Exp/Ln/Copy onto the single combined act table set so the
    # compiler never inserts per-call ACT_TABLE_LOADs between the phase-2
    # exps and the Ln/Exp reciprocal (keeps act_func_set ids truthful:
    # dict order is unchanged, other sets just lose the overlapping funcs).
    from concourse.hw_specs import get_activation_tables as _gat

    def _patched_tables(arch):
        tabs = _gat(arch)
        key = "natural_log_exp_and_others"
        comb = tabs[key]
        return {n: (s if n == key else (s - comb)) for n, s in tabs.items()}

    _orig_gat = bacc.get_activation_tables
    bacc.get_activation_tables = _patched_tables

    nc = bacc.Bacc("TRN2", target_bir_lowering=False, debug=False,
                   num_devices=NCORES)
    xT = nc.dram_tensor("xT", [D, T], BF16, kind="ExternalInput").ap()
    wqkvT = nc.dram_tensor("wqkvT", [D, 384], BF16, kind="ExternalInput").ap()
    woT = nc.dram_tensor("woT", [256, D], BF16, kind="ExternalInput").ap()
    c4 = nc.dram_tensor("c4", [128, S], BF16, kind="ExternalInput").ap()
    s4 = nc.dram_tensor("s4", [128, S], BF16, kind="ExternalInput").ap()
    maskP = nc.dram_tensor("maskP", [128, 2 * 4 * 512], BF16,
                           kind="ExternalInput").ap()
    o = nc.dram_tensor("o", [T, D], BF16, kind="ExternalOutput").ap()

    with tile.TileContext(nc) as tc:
        with tc.tile_pool(name="resident", bufs=1) as res:
            ident64 = res.tile([64, 64], BF16)
            make_identity(nc, ident64[:])
            c4_sb = res.tile([128, S], BF16)
            s4_sb = res.tile([128, S], BF16)
            maskP_sb = res.tile([128, 2 * 4 * 512], BF16)
            nc.sync.dma_start(c4_sb[:], c4[:])
            nc.sync.dma_start(s4_sb[:], s4[:])
            nc.sync.dma_start(maskP_sb[:], maskP[:])

            QRI_A = res.tile([128, T], BF16)   # [h0r h0i h1r h1i] x tokens
            QRI_B = res.tile([128, T], BF16)   # [h2r h2i h3r h3i]
            KRI2 = res.tile([128, T], BF16)    # [Kr Ki Kr Ki]
            Vt_sb = res.tile([128, 32 * 65], BF16)  # kt-tile k at cols k*65
            Vt3 = Vt_sb.rearrange("p (k c) -> p k c", c=65)
            wqkv_r = res.tile([128, 16 * 384], BF16)
            woT_r = res.tile([128, 2 * D], BF16)
            ones32 = res.tile([128, 32], BF16)
            nc.gpsimd.memset(ones32[:], 1.0)
            nc.vector.tensor_copy(Vt3[:, :, 64], ones32[:])
            dbg_pg = (res.tile([128, 1024], BF16, name="dbg_pg")
                      if phases == 3 else None)

            for d in range(16):
                nc.sync.dma_start(wqkv_r[:, d * 384:(d + 1) * 384],
                                  wqkvT[d * 128:(d + 1) * 128, :])
            for t in range(2):
                nc.sync.dma_start(woT_r[:, t * D:(t + 1) * D],
                                  woT[t * 128:(t + 1) * 128, :])

            # ---------------- phase 1: xT DMA, proj, rope -------------------
            with tc.tile_pool(name="xtp", bufs=3) as xtp, \
                 tc.tile_pool(name="ropet", bufs=2) as rp, \
                 tc.tile_pool(name="vtps", bufs=1, space="PSUM") as vtp_pool, \
                 tc.tile_pool(name="projps", bufs=1, space="PSUM") as projp:

                vsb_prev = None

                def emit_vt(jj, vsb):
                    vtp = vtp_pool.tile([128, 256], BF16, name="vtp")
                    for i in range(4):
                        nc.tensor.transpose(
                            vtp[:, i * 64:(i + 1) * 64],
                            vsb[:, i * 128:(i + 1) * 128], ident64[:])
                    vtp3 = vtp.rearrange("p (k c) -> p k c", c=64)
                    nc.vector.tensor_copy(
                        Vt3[:, jj * 4:jj * 4 + 4, 0:64], vtp3[:])

                for j in range(8):           # 512-token chunks
                    xts = []
                    for d in range(16):
                        xt = xtp.tile([128, 512], BF16, name=f"xt{d}")
                        nc.sync.dma_start(
                            xt[:], xT[d * 128:(d + 1) * 128,
                                      j * 512:(j + 1) * 512])
                        xts.append(xt)
                    QRp = projp.tile([128, 512], F32, name="QRp", bufs=2)
                    QIp = projp.tile([128, 512], F32, name="QIp", bufs=2)
                    KVp = projp.tile([128, 512], F32, name="KVp", bufs=2)
                    for d in range(16):
                        for ch, ps in enumerate((QRp, QIp, KVp)):
                            nc.tensor.matmul(
                                ps[:],
                                wqkv_r[:, d * 384 + ch * 128:
                                       d * 384 + (ch + 1) * 128],
                                xts[d][:], start=(d == 0), stop=(d == 15))
                        if d == 8 and vsb_prev is not None:
                            emit_vt(j - 1, vsb_prev)
                    # rope: ACT stages psum->bf16 SBUF, DVE does bf16 TT
                    # at 2x; KVp released by the kvb copy immediately.
                    tb = j * 512
                    bc = (j % 4) * 512
                    cs = c4_sb[:, bc:bc + 512]
                    sn = s4_sb[:, bc:bc + 512]
                    cs32 = c4_sb[0:32, bc:bc + 512]
                    sn32 = s4_sb[0:32, bc:bc + 512]
                    kb = rp.tile([64, 512], BF16, name="kb")
                    vsb = rp.tile([64, 512], BF16, name="vsb")
                    qrb = rp.tile([128, 512], BF16, name="qrb")
                    qib = rp.tile([128, 512], BF16, name="qib")
                    nc.scalar.copy(kb[:], KVp[0:64, :])
                    nc.scalar.copy(vsb[:], KVp[64:128, :])
                    nc.scalar.copy(qrb[:], QRp[:])
                    nc.scalar.copy(qib[:], QIp[:])
                    u1 = rp.tile([32, 512], BF16, name="u1", bufs=1)
                    u2 = rp.tile([32, 512], BF16, name="u2", bufs=1)
                    u3 = rp.tile([32, 512], BF16, name="u3", bufs=1)
                    u4 = rp.tile([32, 512], BF16, name="u4", bufs=1)
                    cs32b = c4_sb[32:64, bc:bc + 512]
                    sn32b = s4_sb[32:64, bc:bc + 512]
                    nc.vector.tensor_mul(u1[:], kb[0:32, :], cs32)
                    nc.vector.tensor_mul(u2[:], kb[32:64, :], sn32b)
                    nc.vector.tensor_mul(u3[:], kb[0:32, :], sn32)
                    nc.vector.tensor_mul(u4[:], kb[32:64, :], cs32b)
                    for g in (0, 64):
                        nc.vector.tensor_sub(
                            KRI2[g:g + 32, tb:tb + 512], u1[:], u2[:])
                    for g in (32, 96):
                        nc.vector.tensor_add(
                            KRI2[g:g + 32, tb:tb + 512], u3[:], u4[:])
                    t1 = rp.tile([128, 512], BF16, name="t1", bufs=1)
                    t2 = rp.tile([128, 512], BF16, name="t2", bufs=1)
                    t3 = rp.tile([128, 512], BF16, name="t3", bufs=1)
                    t4 = rp.tile([128, 512], BF16, name="t4", bufs=1)
                    nc.vector.tensor_mul(t1[:], qrb[:], cs)
                    nc.vector.tensor_mul(t3[:], qrb[:], sn)
                    nc.vector.tensor_mul(t2[:], qib[:], sn)
                    nc.vector.tensor_mul(t4[:], qib[:], cs)
                    for hh in range(4):
                        dst = QRI_A if hh < 2 else QRI_B
                        base = (hh % 2) * 64
                        nc.vector.tensor_sub(
                            dst[base:base + 32, tb:tb + 512],
                            t1[32 * hh:32 * hh + 32, :],
                            t2[32 * hh:32 * hh + 32, :])
                        nc.vector.tensor_add(
                            dst[base + 32:base + 64, tb:tb + 512],
                            t3[32 * hh:32 * hh + 32, :],
                            t4[32 * hh:32 * hh + 32, :])
                    vsb_prev = vsb
                emit_vt(7, vsb_prev)

            dbg = {}
            if phases < 2:
                nc.sync.dma_start(o[0:128, :], QRI_A[:, 0:2048])
            # -------------- phase 2: attention + wo, per qt-512 pair --------
            else:
                with tc.tile_pool(name="probs", bufs=3) as probsp, \
                     tc.tile_pool(name="attnp", bufs=2) as attnp, \
                     tc.tile_pool(name="normp", bufs=2) as normp, \
                     tc.tile_pool(name="outp", bufs=2) as outp, \
                     tc.tile_pool(name="sps", bufs=2, space="PSUM") as sps, \
                     tc.tile_pool(name="pvps", bufs=1, space="PSUM") as pvps, \
                     tc.tile_pool(name="ops", bufs=2, space="PSUM") as opsp:

                    pending = deque()

                    def enqueue_wo(attn01, attn23, qb):
                        state = {}
                        for qs in range(4):
                            for do in range(4):
                                pending.append(
                                    (attn01, attn23, qb, qs, do, state))

                    def drain_wo(n):
                        for _ in range(n):
                            if not pending:
                                return
                            attn01, attn23, qb, qs, do, state = \
                                pending.popleft()
                            if do == 0:
                                state[qs] = outp.tile([128, D], BF16,
                                                      name="osb")
                            osb = state[qs]
                            Ops = opsp.tile([128, 512], F32, name="Ops")
                            nc.tensor.matmul(
                                Ops[:], attn01[:, qs * 128:(qs + 1) * 128],
                                woT_r[:, do * 512:(do + 1) * 512],
                                start=True, stop=False)
                            nc.tensor.matmul(
                                Ops[:], attn23[:, qs * 128:(qs + 1) * 128],
                                woT_r[:, D + do * 512:D + (do + 1) * 512],
                                start=False, stop=True)
                            nc.vector.tensor_copy(
                                osb[:, do * 512:(do + 1) * 512], Ops[:])
                            if do == 3 and phases != 3:
                                qq = qb + qs * 128
                                nc.sync.dma_start(o[qq:qq + 128, :], osb[:])

                    def emit_pv(PVs, b, pkt, ppg, pcsl, stop):
                        vt = Vt3[:, b * 16 + pkt, :]
                        st = (pkt == 0)
                        for hh in range(2):
                            hs = hh * 512
                            nc.tensor.matmul(
                                PVs[:, hs + pcsl.start:hs + pcsl.stop], vt,
                                ppg[:, hs + pcsl.start:hs + pcsl.stop],
                                start=st, stop=stop)

                    for b in range(2):
                        for jp in range(4):          # qt-512 pairs
                            qb = b * S + jp * 512
                            nkt = 4 * jp + 4
                            attn01 = attnp.tile([128, 512], BF16, name="at01")
                            attn23 = attnp.tile([128, 512], BF16, name="at23")
                            pvc = normp.tile([65, 2048], F32, name="pvc")
                            for pi, (QRI, attn) in enumerate(
                                    ((QRI_A, attn01), (QRI_B, attn23))):
                                PVs = pvps.tile([65, 1024], F32, name="PV")
                                pg_prev = None
                                for kt in range(nkt):
                                    kc = b * S + kt * 128
                                    r = kt - (nkt - 4)
                                    half = r >= 2   # only right half live
                                    csl = slice(256, 512) if half \
                                        else slice(0, 512)
                                    Sg = sps.tile([128, 1024], F32, name="Sg")
                                    pg = probsp.tile([128, 1024], BF16,
                                                     name="pg")
                                    for hh in range(2):
                                        hs = hh * 512
                                        nc.tensor.matmul(
                                            Sg[:, hs + csl.start:
                                               hs + csl.stop],
                                            KRI2[64 * hh:64 * hh + 64,
                                                 kc:kc + 128],
                                            QRI[64 * hh:64 * hh + 64,
                                                qb + csl.start:
                                                qb + csl.stop],
                                            start=True, stop=True,
                                            tile_position=(64 * hh, 0))
                                    if half:
                                        sgv = Sg.rearrange(
                                            "p (h c) -> p h c",
                                            h=2)[:, :, 256:512]
                                        pgv = pg.rearrange(
                                            "p (h c) -> p h c",
                                            h=2)[:, :, 256:512]
                                        nc.scalar.activation(
                                            pgv, sgv, AF.Exp, scale=0.125)
                                    else:
                                        nc.scalar.activation(
                                            pg[:], Sg[:], AF.Exp, scale=0.125)
                                    if r >= 0:
                                        if half:
                                            msl = slice(512 * r + 256,
                                                        512 * r + 512)
                                            psl = slice(256, 512)
                                        else:
                                            msl = slice(512 * r,
                                                        512 * r + 256)
                                            psl = slice(0, 256)
                                        pgv = pg.rearrange(
                                            "p (h c) -> p h c", h=2)[:, :, psl]
                                        mkv = maskP_sb.rearrange(
                                            "p (h c) -> p h c", h=2)[:, :, msl]
                                        nc.vector.tensor_mul(pgv, pgv, mkv)
                                    if pg_prev is not None:
                                        emit_pv(PVs, b, *pg_prev, stop=False)
                                    pg_prev = (kt, pg, csl)
                                    if (phases == 3 and b == 1 and jp == 3
                                            and pi == 1 and kt == nkt - 4):
                                        nc.vector.tensor_copy(dbg_pg[:],
                                                              pg[:])
                                    drain_wo(1)
                                emit_pv(PVs, b, *pg_prev, stop=True)
                                # free the PV banks asap; norm happens once
                                # per pair, off the PE critical path
                                nc.vector.tensor_copy(
                                    pvc[:, 1024 * pi:1024 * pi + 1024],
                                    PVs[:])
                            lnd = normp.tile([1, 2048], F32, name="lnd")
                            nc.scalar.activation(
                                lnd[:], pvc[64:65, :], AF.Ln)
                            rec = normp.tile([1, 2048], F32, name="rec")
                            nc.scalar.activation(
                                rec[:], lnd[:], AF.Exp, scale=-1.0)
                            bcst = normp.tile([64, 2048], F32, name="bc")
                            nc.gpsimd.partition_broadcast(bcst[:], rec[:])
                            for pi, attn in enumerate((attn01, attn23)):
                                for hh in range(2):
                                    cs_ = 1024 * pi + 512 * hh
                                    nc.vector.tensor_mul(
                                        attn[64 * hh:64 * hh + 64, :],
                                        pvc[0:64, cs_:cs_ + 512],
                                        bcst[:, cs_:cs_ + 512])
                            if b == 1 and jp == 3:
                                dbg.update(pvc=pvc, rec=rec, bcst=bcst)
                            enqueue_wo(attn01, attn23, qb)
                            if b == 1 and jp == 3:
                                dbg.update(at01=attn01, at23=attn23)
                    drain_wo(1 << 30)
                    if phases == 3:
                        # debug dumps into sacrificial o rows (bf16)
                        scr = normp.tile([128, 2048], BF16, name="dscr")
                        nc.gpsimd.memset(scr[:], 0.0)
                        nc.sync.dma_start(o[0:128, :], QRI_A[:, 0:2048])
                        nc.sync.dma_start(o[128:256, :], KRI2[:, 0:2048])
                        nc.sync.dma_start(o[256:384, :], Vt_sb[:, 0:2048])
                        nc.vector.tensor_copy(scr[0:65, :], dbg["pvc"][:])
                        nc.sync.dma_start(o[384:512, :], scr[:])
                        scr4 = normp.tile([64, 2048], BF16, name="dscr4")
                        nc.vector.tensor_copy(scr4[:], dbg["bcst"][:])
                        nc.sync.dma_start(o[768:832, :], scr4[:])
                        nc.sync.dma_start(o[512:640, 0:1024], dbg_pg[:])
                        scr3 = normp.tile([128, 2048], BF16, name="dscr3")
                        nc.vector.tensor_copy(scr3[:, 0:512], dbg["at01"][:])
                        nc.vector.tensor_copy(scr3[:, 512:1024],
                                              dbg["at23"][:])
                        nc.sync.dma_start(o[640:768, 0:1024],
                                          scr3[:, 0:1024])

    nc.compile()
    bacc.get_activation_tables = _orig_gat
    return nc


def _prep_inputs(x, freqs_cos, freqs_sin, wq, wk, wv, wo):
    from ml_dtypes import bfloat16
    xf = np.asarray(x, np.float32).reshape(T, D)
    xTf = np.ascontiguousarray(xf.T).astype(bfloat16)      # [D, T]
    wq = np.asarray(wq, np.float32)
    wk = np.asarray(wk, np.float32)
    wv = np.asarray(wv, np.float32)
    wo = np.asarray(wo, np.float32)
    fc = np.asarray(freqs_cos, np.float32)
    fs = np.asarray(freqs_sin, np.float32)

    c4 = np.ascontiguousarray(np.tile(fc.T, (4, 1))).astype(bfloat16)
    s4 = np.ascontiguousarray(np.tile(fs.T, (4, 1))).astype(bfloat16)
    kt = np.arange(128)[:, None]
    qt = np.arange(256)[None, :]
    mA = (kt <= qt).astype(np.float32)
    mB = (kt + 128 <= qt).astype(np.float32)
    one = np.ones((128, 256), np.float32)
    zero = np.zeros((128, 256), np.float32)
    maskP1 = np.concatenate([
        np.concatenate([mA, one], axis=1),
        np.concatenate([mB, one], axis=1),
        np.concatenate([zero, mA], axis=1),
        np.concatenate([zero, mB], axis=1)], axis=1)
    maskP = np.ascontiguousarray(
        np.tile(maskP1, (1, 2))).astype(bfloat16)      # [128, 4096]
    ev = np.arange(0, 64, 2)
    od = np.arange(1, 64, 2)

    in_maps = []
    for c in range(NCORES):
        qreal = np.concatenate([(4 * c + h) * 64 + ev for h in range(4)])
        qimag = np.concatenate([(4 * c + h) * 64 + od for h in range(4)])
        Wc = np.concatenate([wq[qreal], wq[qimag], wk[c * 64 + ev],
                             wk[c * 64 + od], wv[c * 64:(c + 1) * 64]], axis=0)
        in_maps.append({
            "xT": xTf,
            "wqkvT": np.ascontiguousarray(Wc.T).astype(bfloat16),
            "woT": np.ascontiguousarray(
                wo[:, c * 256:(c + 1) * 256].T).astype(bfloat16),
            "c4": c4, "s4": s4, "maskP": maskP,
        })
    return in_maps


def _run(in_maps, trace=False, **kw):
    from concourse import bass_utils
    if "nc" not in _cache:
        _cache["nc"] = _build()
    return bass_utils.run_bass_kernel_spmd(
        _cache["nc"], in_maps, core_ids=list(range(NCORES)), trace=trace, **kw)


def kernel(x, freqs_cos, freqs_sin, wq, wk, wv, wo):
    in_maps = _prep_inputs(x, freqs_cos, freqs_sin, wq, wk, wv, wo)
    res = _run(in_maps)
    out = np.zeros((T, D), np.float64)
    for c in range(NCORES):
        out += np.asarray(res.results[c]["o"], np.float32)
    return out.astype(np.float32).reshape(B, S, D)


# revision 45
# speedup vs baseline: 1.9635x; 1.0214x over previous
"""Trainium2 Bass kernel for nn_Attention_14542759264705.

Dense transformer attention: QKV proj + interleaved RoPE + GQA causal
attention (32 q heads / 8 kv heads, hd=64) + output proj, fp32 in/out.

Sharding: tensor-parallel over kv-head groups across 8 cores. Core c owns
q heads 4c..4c+3 and kv head c; each core computes a partial output and
the host sums the 8 partials.

v2 (vs the fp32r baseline):
  - x is transposed on the HOST (xT input) -> no PE transposes / ACT
    copies for the projection's moving operand.
  - All matmul operands are bf16 (PSUM accumulation stays f32): halves
    DMA volume, enables fast weight load, 2x DVE on 16-bit tiles. fp32r
    at N>=256 is already 1 cyc/row, so MM time is unchanged; the wins
    are bandwidth + LDWEIGHTS + elementwise.
  - The two per-head-pair exps are merged into one [128,1024] ACT call
    (amortizes the ~352-cycle ACT fixed cost; ACT exp is the phase-2
    pace-setter).
  - Softmax normalization: DVE reciprocal_approx_fast on the [1,1024]
    denominator row (the old nc.vector.reciprocal was 8 cyc/elem),
    gpsimd partition-broadcast, DVE muls. All off the PE critical path.
  - wo matmuls are drip-fed one (qs,do) step per kt-iteration into the
    NEXT pair's attention loop so the PE never parks while ACT works,
    instead of a 7us wo burst that starves ACT.
"""
import numpy as np

B, S, D = 2, 2048, 2048
T = B * S
NH, NKV, HD = 32, 8, 64
NCORES = 8

_cache = {}


def _build(phases=99):
    from collections import deque

    import concourse.bacc as bacc
    import concourse.mybir as mybir
    import concourse.tile as tile
    from concourse.masks import make_identity

    F32 = mybir.dt.float32
    BF16 = mybir.dt.bfloat16
    AF = mybir.ActivationFunctionType

    # Force Exp/Ln/Copy onto the single combined act table set so the
    # compiler never inserts per-call ACT_TABLE_LOADs between the phase-2
    # exps and the Ln/Exp reciprocal (keeps act_func_set ids truthful:
    # dict order is unchanged, other sets just lose the overlapping funcs).
    from concourse.hw_specs import get_activation_tables as _gat

    def _patched_tables(arch):
        tabs = _gat(arch)
        key = "natural_log_exp_and_others"
        comb = tabs[key]
        return {n: (s if n == key else (s - comb)) for n, s in tabs.items()}

    _orig_gat = bacc.get_activation_tables
    bacc.get_activation_tables = _patched_tables

    nc = bacc.Bacc("TRN2", target_bir_lowering=False, debug=False,
                   num_devices=NCORES)
    xT = nc.dram_tensor("xT", [D, T], BF16, kind="ExternalInput").ap()
    wqkvT = nc.dram_tensor("wqkvT", [D, 384], BF16, kind="ExternalInput").ap()
    woT = nc.dram_tensor("woT", [256, D], BF16, kind="ExternalInput").ap()
    c4 = nc.dram_tensor("c4", [128, S], BF16, kind="ExternalInput").ap()
    s4 = nc.dram_tensor("s4", [128, S], BF16, kind="ExternalInput").ap()
    maskP = nc.dram_tensor("maskP", [128, 2 * 4 * 512], BF16,
                           kind="ExternalInput").ap()
    o = nc.dram_tensor("o", [T, D], BF16, kind="ExternalOutput").ap()

    with tile.TileContext(nc) as tc:
        with tc.tile_pool(name="resident", bufs=1) as res:
            ident64 = res.tile([64, 64], BF16)
            make_identity(nc, ident64[:])
            c4_sb = res.tile([128, S], BF16)
            s4_sb = res.tile([128, S], BF16)
            maskP_sb = res.tile([128, 2 * 4 * 512], BF16)
            nc.sync.dma_start(c4_sb[:], c4[:])
            nc.sync.dma_start(s4_sb[:], s4[:])
            nc.sync.dma_start(maskP_sb[:], maskP[:])

            QRI_A = res.tile([128, T], BF16)   # [h0r h0i h1r h1i] x tokens
            QRI_B = res.tile([128, T], BF16)   # [h2r h2i h3r h3i]
            KRI2 = res.tile([128, T], BF16)    # [Kr Ki Kr Ki]
            Vt_sb = res.tile([128, 32 * 65], BF16)  # kt-tile k at cols k*65
            Vt3 = Vt_sb.rearrange("p (k c) -> p k c", c=65)
            wqkv_r = res.tile([128, 16 * 384], BF16)
            woT_r = res.tile([128, 2 * D], BF16)
            ones32 = res.tile([128, 32], BF16)
            nc.gpsimd.memset(ones32[:], 1.0)
            nc.vector.tensor_copy(Vt3[:, :, 64], ones32[:])
            dbg_pg = (res.tile([128, 1024], BF16, name="dbg_pg")
                      if phases == 3 else None)

            for d in range(16):
                nc.sync.dma_start(wqkv_r[:, d * 384:(d + 1) * 384],
                                  wqkvT[d * 128:(d + 1) * 128, :])
            for t in range(2):
                nc.sync.dma_start(woT_r[:, t * D:(t + 1) * D],
                                  woT[t * 128:(t + 1) * 128, :])

            # ---------------- phase 1: xT DMA, proj, rope -------------------
            with tc.tile_pool(name="xtp", bufs=3) as xtp, \
                 tc.tile_pool(name="ropet", bufs=2) as rp, \
                 tc.tile_pool(name="vtps", bufs=1, space="PSUM") as vtp_pool, \
                 tc.tile_pool(name="projps", bufs=1, space="PSUM") as projp:

                vsb_prev = None

                def emit_vt(jj, vsb):
                    vtp = vtp_pool.tile([128, 256], BF16, name="vtp")
                    for i in range(4):
                        nc.tensor.transpose(
                            vtp[:, i * 64:(i + 1) * 64],
                            vsb[:, i * 128:(i + 1) * 128], ident64[:])
                    vtp3 = vtp.rearrange("p (k c) -> p k c", c=64)
                    nc.vector.tensor_copy(
                        Vt3[:, jj * 4:jj * 4 + 4, 0:64], vtp3[:])

                for j in range(8):           # 512-token chunks
                    xts = []
                    for d in range(16):
                        xt = xtp.tile([128, 512], BF16, name=f"xt{d}")
                        nc.sync.dma_start(
                            xt[:], xT[d * 128:(d + 1) * 128,
                                      j * 512:(j + 1) * 512])
                        xts.append(xt)
                    QRp = projp.tile([128, 512], F32, name="QRp", bufs=2)
                    QIp = projp.tile([128, 512], F32, name="QIp", bufs=2)
                    KVp = projp.tile([128, 512], F32, name="KVp", bufs=2)
                    for d in range(16):
                        for ch, ps in enumerate((QRp, QIp, KVp)):
                            nc.tensor.matmul(
                                ps[:],
                                wqkv_r[:, d * 384 + ch * 128:
                                       d * 384 + (ch + 1) * 128],
                                xts[d][:], start=(d == 0), stop=(d == 15))
                        if d == 8 and vsb_prev is not None:
                            emit_vt(j - 1, vsb_prev)
                    # rope: ACT stages psum->bf16 SBUF, DVE does bf16 TT
                    # at 2x; KVp released by the kvb copy immediately.
                    tb = j * 512
                    bc = (j % 4) * 512
                    cs = c4_sb[:, bc:bc + 512]
                    sn = s4_sb[:, bc:bc + 512]
                    cs32 = c4_sb[0:32, bc:bc + 512]
                    sn32 = s4_sb[0:32, bc:bc + 512]
                    kb = rp.tile([64, 512], BF16, name="kb")
                    vsb = rp.tile([64, 512], BF16, name="vsb")
                    qrb = rp.tile([128, 512], BF16, name="qrb")
                    qib = rp.tile([128, 512], BF16, name="qib")
                    nc.scalar.copy(kb[:], KVp[0:64, :])
                    nc.scalar.copy(vsb[:], KVp[64:128, :])
                    nc.scalar.copy(qrb[:], QRp[:])
                    nc.scalar.copy(qib[:], QIp[:])
                    u1 = rp.tile([32, 512], BF16, name="u1", bufs=1)
                    u2 = rp.tile([32, 512], BF16, name="u2", bufs=1)
                    u3 = rp.tile([32, 512], BF16, name="u3", bufs=1)
                    u4 = rp.tile([32, 512], BF16, name="u4", bufs=1)
                    cs32b = c4_sb[32:64, bc:bc + 512]
                    sn32b = s4_sb[32:64, bc:bc + 512]
                    nc.vector.tensor_mul(u1[:], kb[0:32, :], cs32)
                    nc.vector.tensor_mul(u2[:], kb[32:64, :], sn32b)
                    nc.vector.tensor_mul(u3[:], kb[0:32, :], sn32)
                    nc.vector.tensor_mul(u4[:], kb[32:64, :], cs32b)
                    for g in (0, 64):
                        nc.vector.tensor_sub(
                            KRI2[g:g + 32, tb:tb + 512], u1[:], u2[:])
                    for g in (32, 96):
                        nc.vector.tensor_add(
                            KRI2[g:g + 32, tb:tb + 512], u3[:], u4[:])
                    t1 = rp.tile([128, 512], BF16, name="t1", bufs=1)
                    t2 = rp.tile([128, 512], BF16, name="t2", bufs=1)
                    t3 = rp.tile([128, 512], BF16, name="t3", bufs=1)
                    t4 = rp.tile([128, 512], BF16, name="t4", bufs=1)
                    nc.vector.tensor_mul(t1[:], qrb[:], cs)
                    nc.vector.tensor_mul(t3[:], qrb[:], sn)
                    nc.vector.tensor_mul(t2[:], qib[:], sn)
                    nc.vector.tensor_mul(t4[:], qib[:], cs)
                    for hh in range(4):
                        dst = QRI_A if hh < 2 else QRI_B
                        base = (hh % 2) * 64
                        nc.vector.tensor_sub(
                            dst[base:base + 32, tb:tb + 512],
                            t1[32 * hh:32 * hh + 32, :],
                            t2[32 * hh:32 * hh + 32, :])
                        nc.vector.tensor_add(
                            dst[base + 32:base + 64, tb:tb + 512],
                            t3[32 * hh:32 * hh + 32, :],
                            t4[32 * hh:32 * hh + 32, :])
                    vsb_prev = vsb
                emit_vt(7, vsb_prev)

            dbg = {}
            if phases < 2:
                nc.sync.dma_start(o[0:128, :], QRI_A[:, 0:2048])
            # -------------- phase 2: attention + wo, per qt-512 pair --------
            else:
                with tc.tile_pool(name="probs", bufs=3) as probsp, \
                     tc.tile_pool(name="attnp", bufs=2) as attnp, \
                     tc.tile_pool(name="normp", bufs=2) as normp, \
                     tc.tile_pool(name="outp", bufs=2) as outp, \
                     tc.tile_pool(name="sps", bufs=2, space="PSUM") as sps, \
                     tc.tile_pool(name="pvps", bufs=1, space="PSUM") as pvps, \
                     tc.tile_pool(name="ops", bufs=2, space="PSUM") as opsp:

                    pending = deque()
                    norm_q = deque()
                    nprog = {"enq": 0, "done": 0}

                    def enqueue_wo(attn01, attn23, qb):
                        state = {}
                        pid = nprog["enq"]
                        nprog["enq"] += 1
                        for qs in range(4):
                            for do in range(4):
                                pending.append(
                                    (pid, attn01, attn23, qb, qs, do, state))

                    def drain_wo(n):
                        for _ in range(n):
                            if not pending:
                                return
                            if pending[0][0] >= nprog["done"]:
                                return   # this pair's attn not normed yet
                            pid, attn01, attn23, qb, qs, do, state = \
                                pending.popleft()
                            if do == 0:
                                state[qs] = outp.tile([128, D], BF16,
                                                      name="osb")
                            osb = state[qs]
                            Ops = opsp.tile([128, 512], F32, name="Ops")
                            nc.tensor.matmul(
                                Ops[:], attn01[:, qs * 128:(qs + 1) * 128],
                                woT_r[:, do * 512:(do + 1) * 512],
                                start=True, stop=False)
                            nc.tensor.matmul(
                                Ops[:], attn23[:, qs * 128:(qs + 1) * 128],
                                woT_r[:, D + do * 512:D + (do + 1) * 512],
                                start=False, stop=True)
                            nc.vector.tensor_copy(
                                osb[:, do * 512:(do + 1) * 512], Ops[:])
                            if do == 3 and phases != 3:
                                qq = qb + qs * 128
                                nc.sync.dma_start(o[qq:qq + 128, :], osb[:])

                    def emit_pv(PVs, b, pkt, ppg, pcsl, stop):
                        vt = Vt3[:, b * 16 + pkt, :]
                        st = (pkt == 0)
                        for hh in range(2):
                            hs = hh * 512
                            nc.tensor.matmul(
                                PVs[:, hs + pcsl.start:hs + pcsl.stop], vt,
                                ppg[:, hs + pcsl.start:hs + pcsl.stop],
                                start=st, stop=stop)

                    for b in range(2):
                        for jp in range(4):          # qt-512 pairs
                            qb = b * S + jp * 512
                            nkt = 4 * jp + 4
                            attn01 = attnp.tile([128, 512], BF16, name="at01")
                            attn23 = attnp.tile([128, 512], BF16, name="at23")
                            pvc = normp.tile([65, 2048], F32, name="pvc")
                            for pi, (QRI, attn) in enumerate(
                                    ((QRI_A, attn01), (QRI_B, attn23))):
                                PVs = pvps.tile([65, 1024], F32, name="PV")
                                pg_prev = None
                                for kt in range(nkt):
                                    kc = b * S + kt * 128
                                    r = kt - (nkt - 4)
                                    half = r >= 2   # only right half live
                                    csl = slice(256, 512) if half \
                                        else slice(0, 512)
                                    Sg = sps.tile([128, 1024], F32, name="Sg")
                                    pg = probsp.tile([128, 1024], BF16,
                                                     name="pg")
                                    for hh in range(2):
                                        hs = hh * 512
                                        nc.tensor.matmul(
                                            Sg[:, hs + csl.start:
                                               hs + csl.stop],
                                            KRI2[64 * hh:64 * hh + 64,
                                                 kc:kc + 128],
                                            QRI[64 * hh:64 * hh + 64,
                                                qb + csl.start:
                                                qb + csl.stop],
                                            start=True, stop=True,
                                            tile_position=(64 * hh, 0))
                                    if half:
                                        sgv = Sg.rearrange(
                                            "p (h c) -> p h c",
                                            h=2)[:, :, 256:512]
                                        pgv = pg.rearrange(
                                            "p (h c) -> p h c",
                                            h=2)[:, :, 256:512]
                                        nc.scalar.activation(
                                            pgv, sgv, AF.Exp, scale=0.125)
                                    else:
                                        nc.scalar.activation(
                                            pg[:], Sg[:], AF.Exp, scale=0.125)
                                    if r >= 0:
                                        if half:
                                            msl = slice(512 * r + 256,
                                                        512 * r + 512)
                                            psl = slice(256, 512)
                                        else:
                                            msl = slice(512 * r,
                                                        512 * r + 256)
                                            psl = slice(0, 256)
                                        pgv = pg.rearrange(
                                            "p (h c) -> p h c", h=2)[:, :, psl]
                                        mkv = maskP_sb.rearrange(
                                            "p (h c) -> p h c", h=2)[:, :, msl]
                                        nc.vector.tensor_mul(pgv, pgv, mkv)
                                    if pg_prev is not None:
                                        emit_pv(PVs, b, *pg_prev, stop=False)
                                    pg_prev = (kt, pg, csl)
                                    if (phases == 3 and b == 1 and jp == 3
                                            and pi == 1 and kt == nkt - 4):
                                        nc.vector.tensor_copy(dbg_pg[:],
                                                              pg[:])
                                    if pi == 0 and kt in (1, 2, 3) and norm_q:
                                        norm_q.popleft()()
                                    drain_wo(1)
                                emit_pv(PVs, b, *pg_prev, stop=True)
                                # free the PV banks asap; norm happens once
                                # per pair, off the PE critical path
                                nc.vector.tensor_copy(
                                    pvc[:, 1024 * pi:1024 * pi + 1024],
                                    PVs[:])
                            # norm stages are deferred into the NEXT pair's
                            # kt loop so the ACT FIFO never stalls on them
                            def make_norm(pvc, attn01, attn23, last):
                                st = {}

                                def s1():
                                    st["lnd"] = normp.tile(
                                        [1, 2048], F32, name="lnd")
                                    nc.scalar.activation(
                                        st["lnd"][:], pvc[64:65, :], AF.Ln)

                                def s2():
                                    st["rec"] = normp.tile(
                                        [1, 2048], F32, name="rec")
                                    nc.scalar.activation(
                                        st["rec"][:], st["lnd"][:],
                                        AF.Exp, scale=-1.0)

                                def s3():
                                    bcst = normp.tile(
                                        [64, 2048], F32, name="bc")
                                    nc.gpsimd.partition_broadcast(
                                        bcst[:], st["rec"][:])
                                    for pi, attn in enumerate(
                                            (attn01, attn23)):
                                        for hh in range(2):
                                            cs_ = 1024 * pi + 512 * hh
                                            nc.vector.tensor_mul(
                                                attn[64 * hh:
                                                     64 * hh + 64, :],
                                                pvc[0:64, cs_:cs_ + 512],
                                                bcst[:, cs_:cs_ + 512])
                                    nprog["done"] += 1
                                    if last:
                                        dbg.update(pvc=pvc, rec=st["rec"],
                                                   bcst=bcst)
                                return [s1, s2, s3]

                            norm_q.extend(make_norm(
                                pvc, attn01, attn23, b == 1 and jp == 3))
                            enqueue_wo(attn01, attn23, qb)
                            if b == 1 and jp == 3:
                                dbg.update(at01=attn01, at23=attn23)
                    while norm_q:
                        norm_q.popleft()()
                    drain_wo(1 << 30)
                    if phases == 3:
                        # debug dumps into sacrificial o rows (bf16)
                        scr = normp.tile([128, 2048], BF16, name="dscr")
                        nc.gpsimd.memset(scr[:], 0.0)
                        nc.sync.dma_start(o[0:128, :], QRI_A[:, 0:2048])
                        nc.sync.dma_start(o[128:256, :], KRI2[:, 0:2048])
                        nc.sync.dma_start(o[256:384, :], Vt_sb[:, 0:2048])
                        nc.vector.tensor_copy(scr[0:65, :], dbg["pvc"][:])
                        nc.sync.dma_start(o[384:512, :], scr[:])
                        scr4 = normp.tile([64, 2048], BF16, name="dscr4")
                        nc.vector.tensor_copy(scr4[:], dbg["bcst"][:])
                        nc.sync.dma_start(o[768:832, :], scr4[:])
                        nc.sync.dma_start(o[512:640, 0:1024], dbg_pg[:])
                        scr3 = normp.tile([128, 2048], BF16, name="dscr3")
                        nc.vector.tensor_copy(scr3[:, 0:512], dbg["at01"][:])
                        nc.vector.tensor_copy(scr3[:, 512:1024],
                                              dbg["at23"][:])
                        nc.sync.dma_start(o[640:768, 0:1024],
                                          scr3[:, 0:1024])

    nc.compile()
    bacc.get_activation_tables = _orig_gat
    return nc


def _prep_inputs(x, freqs_cos, freqs_sin, wq, wk, wv, wo):
    from ml_dtypes import bfloat16
    xf = np.asarray(x, np.float32).reshape(T, D)
    xTf = np.ascontiguousarray(xf.T).astype(bfloat16)      # [D, T]
    wq = np.asarray(wq, np.float32)
    wk = np.asarray(wk, np.float32)
    wv = np.asarray(wv, np.float32)
    wo = np.asarray(wo, np.float32)
    fc = np.asarray(freqs_cos, np.float32)
    fs = np.asarray(freqs_sin, np.float32)

    c4 = np.ascontiguousarray(np.tile(fc.T, (4, 1))).astype(bfloat16)
    s4 = np.ascontiguousarray(np.tile(fs.T, (4, 1))).astype(bfloat16)
    kt = np.arange(128)[:, None]
    qt = np.arange(256)[None, :]
    mA = (kt <= qt).astype(np.float32)
    mB = (kt + 128 <= qt).astype(np.float32)
    one = np.ones((128, 256), np.float32)
    zero = np.zeros((128, 256), np.float32)
    maskP1 = np.concatenate([
        np.concatenate([mA, one], axis=1),
        np.concatenate([mB, one], axis=1),
        np.concatenate([zero, mA], axis=1),
        np.concatenate([zero, mB], axis=1)], axis=1)
    maskP = np.ascontiguousarray(
        np.tile(maskP1, (1, 2))).astype(bfloat16)      # [128, 4096]
    ev = np.arange(0, 64, 2)
    od = np.arange(1, 64, 2)

    in_maps = []
    for c in range(NCORES):
        qreal = np.concatenate([(4 * c + h) * 64 + ev for h in range(4)])
        qimag = np.concatenate([(4 * c + h) * 64 + od for h in range(4)])
        Wc = np.concatenate([wq[qreal], wq[qimag], wk[c * 64 + ev],
                             wk[c * 64 + od], wv[c * 64:(c + 1) * 64]], axis=0)
        in_maps.append({
            "xT": xTf,
            "wqkvT": np.ascontiguousarray(Wc.T).astype(bfloat16),
            "woT": np.ascontiguousarray(
                wo[:, c * 256:(c + 1) * 256].T).astype(bfloat16),
            "c4": c4, "s4": s4, "maskP": maskP,
        })
    return in_maps


def _run(in_maps, trace=False, **kw):
    from concourse import bass_utils
    if "nc" not in _cache:
        _cache["nc"] = _build()
    return bass_utils.run_bass_kernel_spmd(
        _cache["nc"], in_maps, core_ids=list(range(NCORES)), trace=trace, **kw)


def kernel(x, freqs_cos, freqs_sin, wq, wk, wv, wo):
    in_maps = _prep_inputs(x, freqs_cos, freqs_sin, wq, wk, wv, wo)
    res = _run(in_maps)
    out = np.zeros((T, D), np.float64)
    for c in range(NCORES):
        out += np.asarray(res.results[c]["o"], np.float32)
    return out.astype(np.float32).reshape(B, S, D)
